# revision 1
# baseline (speedup 1.0000x reference)
"""Data-parallel CorrelationalDetector kernel for 8 Trainium2 NeuronCores.

Strategy (per spec sharding_hint): pure data parallel — the batch dim (64)
of crop/frame is sharded 8-ways across the NeuronCores (8 samples/core);
conv weights are replicated. Each core runs the full 5-layer encoder on its
crop and frame shards, then the per-sample cross-correlation. BatchNorm
batch statistics are computed globally after gathering the per-core shards
(the gather IS the all-reduce of per-device stats), and the normalization
uses exact global batch stats, matching the reference bit-for-bit in
distribution.

kernel(**inputs) takes FULL unsharded inputs and returns the FULL output.
"""

import numpy as np
import jax
import jax.numpy as jnp
from jax import lax

# Encoder config: (out_ch, kernel, stride), 3x3 convs, padding=1.
_LAYERS = [(3, 3, 2), (16, 3, 2), (64, 3, 1), (128, 3, 2), (256, 3, 1)]
_DN = ("NCHW", "OIHW", "NCHW")

_N_CORES = 8

_compiled = {}


def _encoder(x, Ws, bs):
    for i, (_oc, _k, s) in enumerate(_LAYERS):
        x = lax.conv_general_dilated(
            x, Ws[i], (s, s), ((1, 1), (1, 1)), dimension_numbers=_DN
        )
        x = x + bs[i][None, :, None, None]
        if i < len(_LAYERS) - 1:
            x = jax.nn.relu(x)
    return x


def _shard_fn(crop, frame, W0, b0, W1, b1, W2, b2, W3, b3, W4, b4):
    """Per-core work: encoders + per-sample cross-correlation.

    crop:  [B_local, 3, 64, 64]   -> crop_fm  [B, 256, 8, 8]
    frame: [B_local, 3, 256, 256] -> frame_fm [B, 256, 32, 32]
    returns rmap_local [B_local, 1, 25, 25] (pre-BatchNorm) and local
    (sum, sumsq, count) partial stats.
    """
    Ws = (W0, W1, W2, W3, W4)
    bs = (b0, b1, b2, b3, b4)
    crop_fm = _encoder(crop, Ws, bs)
    frame_fm = _encoder(frame, Ws, bs)

    def xcorr(f, k):  # f:[C,H,W], k:[C,h,w] -> [1,Hr,Wr]
        return lax.conv_general_dilated(
            f[None], k[None], (1, 1), "VALID", dimension_numbers=_DN
        )[0]

    rmap = jax.vmap(xcorr)(frame_fm, crop_fm)  # [B,1,25,25]
    s1 = jnp.sum(rmap)
    s2 = jnp.sum(jnp.square(rmap))
    return rmap, s1, s2


def _get_compiled():
    key = "pmap"
    if key not in _compiled:
        _compiled[key] = jax.pmap(
            _shard_fn,
            axis_name="x",
            in_axes=(0, 0) + (None,) * 10,
            devices=jax.devices()[:_N_CORES],
        )
    return _compiled[key]


def kernel(crop, frame, W0, b0, W1, b1, W2, b2, W3, b3, W4, b4, gamma, beta):
    crop = np.asarray(crop, dtype=np.float32)
    frame = np.asarray(frame, dtype=np.float32)
    B = crop.shape[0]
    bl = B // _N_CORES  # local batch per core

    crop_sh = crop.reshape(_N_CORES, bl, *crop.shape[1:])
    frame_sh = frame.reshape(_N_CORES, bl, *frame.shape[1:])

    f = _get_compiled()
    rmap_sh, s1, s2 = f(
        crop_sh, frame_sh,
        np.asarray(W0, np.float32), np.asarray(b0, np.float32),
        np.asarray(W1, np.float32), np.asarray(b1, np.float32),
        np.asarray(W2, np.float32), np.asarray(b2, np.float32),
        np.asarray(W3, np.float32), np.asarray(b3, np.float32),
        np.asarray(W4, np.float32), np.asarray(b4, np.float32),
    )
    rmap_sh.block_until_ready()

    # Gather/unshard: [8, bl, 1, 25, 25] -> [64, 1, 25, 25]
    rmap = np.asarray(rmap_sh).reshape(B, 1, 25, 25)

    # Global BatchNorm2d(1), training mode: batch stats over (N, H, W).
    # The per-device partial sums are all-reduced here (host-side gather of
    # 8 scalars), giving exact global batch statistics.
    n = float(rmap.size)
    mean = float(np.sum(np.asarray(s1, np.float64)) / n)
    var = float(np.sum(np.asarray(s2, np.float64)) / n) - mean * mean
    g = np.asarray(gamma, np.float32).reshape(1, -1, 1, 1)
    bt = np.asarray(beta, np.float32).reshape(1, -1, 1, 1)
    out = (rmap - np.float32(mean)) * np.float32(1.0 / np.sqrt(var + 1e-5))
    out = out * g + bt
    return out.astype(np.float32)
